# revision 8
# baseline (speedup 1.0000x reference)
"""Causal GQA self-attention (B=4,T=2048,D=1024,H=16,HKV=4) on 8 trn2 cores.

Sharding: core c -> (batch b=c//2, head-half hh=c%2). Each core computes
8 query heads / 2 KV heads for one batch, plus the output projection
restricted to its 512 y-channels (full e). Host sums the two partial
projections per batch.

Kernel pipeline per core (all matmuls bf16, fp32 accumulate):
  x^T via PE transpose -> Q/K/V projections -> RMSnorm+RoPE (natural
  layout, DVE/ACT) -> q^T,k^T via PE transpose (k replicated to both
  partition halves so K=64 score matmuls of a head pair row-pack the
  128x128 array) -> scores^T per 128-key block, wide exp on ACT (no
  max subtraction; scores are bounded), diag-block causal mask ->
  AV with ones-augmented V (softmax denominator comes out as column 64)
  -> per-partition normalize -> y^T via PE transpose -> projection.
"""

import numpy as np

B, T, D = 4, 2048, 1024
H, HKV, HD = 16, 4, 64
P = 128
NT = T // P          # 16 t-tiles
DC = D // P          # 8 contraction chunks
HL = H // 2          # 8 local q heads
PAIRS = HL // 2      # 4 head pairs
ROPE_BASE = 10000.0
EPS = 1.1920928955078125e-07
SCALE = 1.0 / 8.0    # 1/sqrt(HD)

_CACHE = {}


def _rope_tables():
    inv = (1.0 / (ROPE_BASE ** (np.arange(0, HD, 2, dtype=np.float32) / HD))).astype(
        np.float32
    )
    t = np.arange(T, dtype=np.float32)
    f = np.outer(t, inv).astype(np.float32)
    return np.cos(f).astype(np.float32), np.sin(f).astype(np.float32)


def _build_program():
    import concourse.mybir as mybir
    import concourse.tile as tile
    from concourse import bacc
    from concourse.masks import make_identity, make_upper_triangular

    fp32 = mybir.dt.float32
    bf16 = mybir.dt.bfloat16
    AX = mybir.AxisListType.X
    MUL = mybir.AluOpType.mult
    SUB = mybir.AluOpType.subtract
    EXP = mybir.ActivationFunctionType.Exp
    SQRT = mybir.ActivationFunctionType.Sqrt

    nc = bacc.Bacc("TRN2", target_bir_lowering=False, debug=False)

    x_d = nc.dram_tensor("x", [T, D], fp32, kind="ExternalInput").ap()
    wq_d = nc.dram_tensor("wq", [HL * HD, D], fp32, kind="ExternalInput").ap()
    wk_d = nc.dram_tensor("wk", [2 * HD, D], fp32, kind="ExternalInput").ap()
    wv_d = nc.dram_tensor("wv", [2 * HD, D], fp32, kind="ExternalInput").ap()
    wp_d = nc.dram_tensor("wp", [D, HL * HD], fp32, kind="ExternalInput").ap()
    cos_d = nc.dram_tensor("cos", [T, HD // 2], fp32, kind="ExternalInput").ap()
    sin_d = nc.dram_tensor("sin", [T, HD // 2], fp32, kind="ExternalInput").ap()
    gain_d = nc.dram_tensor("gain", [P, HL], fp32, kind="ExternalInput").ap()
    out_d = nc.dram_tensor("out", [T, D], fp32, kind="ExternalOutput").ap()

    x3 = x_d.rearrange("(n p) d -> n p d", p=P)
    out3 = out_d.rearrange("(n p) d -> n p d", p=P)

    with tile.TileContext(nc) as tc:
        with (
            tc.tile_pool(name="persist", bufs=1) as persist,
            tc.tile_pool(name="dram_none", bufs=1) as _,
        ):
            # ---- constants ----
            ident = persist.tile([P, P], bf16)
            make_identity(nc, ident)
            identf = persist.tile([P, P], fp32)
            make_identity(nc, identf)
            dmask = persist.tile([P, P], bf16)
            make_upper_triangular(nc, dmask, val=1.0, diag=True)
            cos_sb = persist.tile([P, NT, HD // 2], bf16)
            sin_sb = persist.tile([P, NT, HD // 2], bf16)
            nc.gpsimd.dma_start(cos_sb, cos_d.rearrange("(n p) c -> p n c", p=P))
            nc.gpsimd.dma_start(sin_sb, sin_d.rearrange("(n p) c -> p n c", p=P))
            gain_sb = persist.tile([P, HL], fp32)
            nc.sync.dma_start(gain_sb, gain_d)
            eps_sb = persist.tile([P, 1], fp32)
            nc.vector.memset(eps_sb, EPS)

            # ---- persistent activations ----
            qT = persist.tile([P, PAIRS, T], bf16)   # [2-head dims, pair, t]
            kT2 = persist.tile([P, 2, T], bf16)      # kv head replicated halves
            v_sb = persist.tile([P, NT, 2 * (HD + 1)], bf16)  # ones-augmented
            y_nat = persist.tile([P, NT, HL * HD], bf16)
            wpT = persist.tile([P, PAIRS, D], bf16)

            # ones columns of v_aug
            v4 = v_sb.rearrange("p n (h x) -> p n h x", h=2)
            nc.gpsimd.memset(v4[:, :, :, HD : HD + 1], 1.0)

            # ================= phase A/B: x^T and weights =================
            with tc.tile_pool(name="xT_pool", bufs=1) as xtp:
                xT = xtp.tile([P, DC, T], bf16)
                wqT = xtp.tile([P, DC, HL * HD], bf16)
                wkT = xtp.tile([P, DC, 2 * HD], bf16)
                wvT = xtp.tile([P, DC, 2 * HD], bf16)

                with (
                    tc.tile_pool(name="stage_ab", bufs=3) as st,
                    tc.tile_pool(name="ps_ab", bufs=4, space="PSUM") as psab,
                ):

                    def transpose_in(src_f32, n_chunk, dst, dst_col0, ncols):
                        # src [P, n_chunk*128] f32 -> dst[:, c, col0:+ncols] bf16
                        for g0 in range(0, n_chunk, 4):
                            gn = min(4, n_chunk - g0)
                            ps = psab.tile([P, 512], fp32, tag="tps")
                            for gi in range(gn):
                                c = g0 + gi
                                nc.tensor.transpose(
                                    ps[:, gi * P : (gi + 1) * P],
                                    src_f32[:, c * P : (c + 1) * P],
                                    identf,
                                )
                            nc.vector.tensor_copy(
                                dst[:, g0 : g0 + gn, dst_col0 : dst_col0 + ncols],
                                ps.rearrange("p (a b) -> p a b", b=P)[:, :gn, :ncols],
                            )

                    for nt in range(NT):
                        xc = st.tile([P, D], fp32, tag="xc")
                        nc.sync.dma_start(xc, x3[nt])
                        transpose_in(xc, DC, xT, nt * P, P)

                    for rt in range(4):  # wq rows: e = rt*128 + p
                        wc = st.tile([P, D], fp32, tag="wc")
                        nc.sync.dma_start(
                            wc, wq_d.rearrange("(r p) d -> r p d", p=P)[rt]
                        )
                        transpose_in(wc, DC, wqT, rt * P, P)
                    wc = st.tile([P, D], fp32, tag="wc")
                    nc.sync.dma_start(wc, wk_d)
                    transpose_in(wc, DC, wkT, 0, P)
                    wc = st.tile([P, D], fp32, tag="wc")
                    nc.sync.dma_start(wc, wv_d)
                    transpose_in(wc, DC, wvT, 0, P)
                    for rt in range(DC):  # wp rows: e = rt*128 + p
                        wc = st.tile([P, HL * HD], fp32, tag="wpc")
                        nc.sync.dma_start(
                            wc, wp_d.rearrange("(r p) d -> r p d", p=P)[rt]
                        )
                        transpose_in(wc, PAIRS, wpT, rt * P, P)

                # ================= phase C: QKV + norm + rope =================
                with (
                    tc.tile_pool(name="stage_c", bufs=3) as sc_st,
                    tc.tile_pool(name="ps_q", bufs=2, space="PSUM") as psq,
                    tc.tile_pool(name="ps_qt", bufs=1, space="PSUM") as psqt,
                    tc.tile_pool(name="ps_kv", bufs=2, space="PSUM") as pskv,
                    tc.tile_pool(name="ps_kt", bufs=1, space="PSUM") as pskt,
                ):
                    for nt in range(NT):
                        q_ps = psq.tile([P, HL * HD], fp32, tag="q")
                        k_ps = pskv.tile([P, 2 * HD], fp32, tag="k")
                        v_ps = pskv.tile([P, 2 * HD], fp32, tag="v")
                        for dc in range(DC):
                            lhs = xT[:, dc, nt * P : (nt + 1) * P]
                            nc.tensor.matmul(
                                q_ps, lhs, wqT[:, dc, :],
                                start=(dc == 0), stop=(dc == DC - 1),
                            )
                            nc.tensor.matmul(
                                k_ps, lhs, wkT[:, dc, :],
                                start=(dc == 0), stop=(dc == DC - 1),
                            )
                            nc.tensor.matmul(
                                v_ps, lhs, wvT[:, dc, :],
                                start=(dc == 0), stop=(dc == DC - 1),
                            )
                        # v: straight copy into augmented layout
                        nc.vector.tensor_copy(
                            v4[:, nt, :, 0:HD],
                            v_ps.rearrange("p (h x) -> p h x", h=2),
                        )

                        def norm_rope(src_ps, nh, gain):
                            # src_ps [P, nh*HD] fp32 psum -> roped+normed bf16
                            sb = sc_st.tile([P, nh * HD], bf16, tag=f"sb{nh}")
                            nc.vector.tensor_copy(sb, src_ps)
                            s3 = sb.rearrange("p (h x) -> p h x", h=nh)
                            sq = sc_st.tile([P, nh * HD], bf16, tag=f"sq{nh}")
                            nc.vector.tensor_tensor(sq, sb, sb, MUL)
                            ss = sc_st.tile([P, nh], fp32, tag=f"ss{nh}")
                            nc.vector.reduce_sum(
                                ss, sq.rearrange("p (h x) -> p h x", h=nh), axis=AX
                            )
                            rms = sc_st.tile([P, nh], fp32, tag=f"rm{nh}")
                            nc.scalar.activation(
                                rms, ss, SQRT, bias=eps_sb[:, 0:1], scale=1.0 / HD
                            )
                            inv = sc_st.tile([P, nh], fp32, tag=f"iv{nh}")
                            nc.vector.reciprocal(inv, rms)
                            if gain is not None:
                                nc.vector.tensor_tensor(inv, inv, gain, MUL)
                            h2 = HD // 2
                            x1 = s3[:, :, 0:h2]
                            x2 = s3[:, :, h2:HD]
                            cb = cos_sb[:, nt : nt + 1, :].to_broadcast([P, nh, h2])
                            sbr = sin_sb[:, nt : nt + 1, :].to_broadcast([P, nh, h2])
                            r = sc_st.tile([P, nh * HD], bf16, tag=f"r{nh}")
                            r3 = r.rearrange("p (h x) -> p h x", h=nh)
                            tmp = sc_st.tile([P, nh * (HD // 2)], bf16, tag=f"t{nh}")
                            t3 = tmp.rearrange("p (h x) -> p h x", h=nh)
                            nc.vector.tensor_tensor(r3[:, :, 0:h2], x1, cb, MUL)
                            nc.vector.tensor_tensor(t3, x2, sbr, MUL)
                            nc.vector.tensor_tensor(
                                r3[:, :, 0:h2], r3[:, :, 0:h2], t3,
                                mybir.AluOpType.add,
                            )
                            nc.vector.tensor_tensor(r3[:, :, h2:HD], x2, cb, MUL)
                            nc.vector.tensor_tensor(t3, x1, sbr, MUL)
                            nc.vector.tensor_tensor(
                                r3[:, :, h2:HD], r3[:, :, h2:HD], t3, SUB
                            )
                            ivb = inv[:, :, None].to_broadcast([P, nh, HD])
                            nc.vector.tensor_tensor(r3, r3, ivb, MUL)
                            return r

                        qr = norm_rope(q_ps, HL, gain_sb)
                        kr = norm_rope(k_ps, 2, None)

                        # q^T: 4 pair transposes
                        ps = psqt.tile([P, 512], bf16, tag="qt")
                        for pr in range(PAIRS):
                            nc.tensor.transpose(
                                ps[:, pr * P : (pr + 1) * P],
                                qr[:, pr * P : (pr + 1) * P],
                                ident,
                            )
                        nc.vector.tensor_copy(
                            qT[:, :, nt * P : (nt + 1) * P],
                            ps.rearrange("p (a b) -> p a b", b=P),
                        )
                        # k^T replicated to both partition halves
                        kps = pskt.tile([P, 2, P], bf16, tag="kt")
                        for kv in range(2):
                            for rep in range(2):
                                nc.tensor.transpose(
                                    kps[rep * 64 : (rep + 1) * 64, kv, :],
                                    kr[:, kv * HD : (kv + 1) * HD],
                                    ident,
                                    tile_position=(0, rep * 64),
                                )
                        nc.vector.tensor_copy(
                            kT2[:, :, nt * P : (nt + 1) * P], kps
                        )

            # ================= phase D: attention =================
            with (
                tc.tile_pool(name="p_pool", bufs=2) as pp,
                tc.tile_pool(name="small", bufs=8) as sm,
                tc.tile_pool(name="ps_sc", bufs=3, space="PSUM") as pssc,
                tc.tile_pool(name="ps_y", bufs=2, space="PSUM") as psy,
            ):
                for pr in range(PAIRS):
                    kv = pr // 2
                    p_tiles = []
                    for tkb in range(NT):
                        w = T - tkb * P
                        pt = pp.tile([P, 2, w], bf16, tag=f"p{tkb}")
                        p_tiles.append(pt)
                        for c0 in range(0, w, 512):
                            cw = min(512, w - c0)
                            sc = pssc.tile([P, 2, 512], fp32, tag="sc")
                            for h01 in range(2):
                                hp = h01 * 64
                                nc.tensor.matmul(
                                    sc[:, h01, :cw],
                                    kT2[hp : hp + 64, kv, tkb * P : (tkb + 1) * P],
                                    qT[hp : hp + 64, pr,
                                       tkb * P + c0 : tkb * P + c0 + cw],
                                    start=True, stop=True,
                                )
                            nc.scalar.activation(
                                pt[:, :, c0 : c0 + cw], sc[:, :, :cw],
                                EXP, scale=SCALE,
                            )
                        # causal mask on the diagonal 128-block
                        dm = dmask[:, None, :].to_broadcast([P, 2, P])
                        nc.vector.tensor_tensor(
                            pt[:, :, 0:P], pt[:, :, 0:P], dm, MUL
                        )
                    for tqi in range(NT):
                        for h01 in range(2):
                            y_ps = psy.tile([P, HD + 1], fp32, tag="y")
                            for tkb in range(tqi + 1):
                                nc.tensor.matmul(
                                    y_ps,
                                    p_tiles[tkb][
                                        :, h01,
                                        (tqi - tkb) * P : (tqi - tkb + 1) * P,
                                    ],
                                    v_sb[:, tkb,
                                         kv * (HD + 1) : (kv + 1) * (HD + 1)],
                                    start=(tkb == 0), stop=(tkb == tqi),
                                )
                            rcp = sm.tile([P, 1], fp32, tag="rcp")
                            nc.vector.reciprocal(rcp, y_ps[:, HD : HD + 1])
                            h = 2 * pr + h01
                            nc.vector.tensor_scalar_mul(
                                y_nat[:, tqi, h * HD : (h + 1) * HD],
                                y_ps[:, 0:HD],
                                rcp,
                            )

            # ================= phase E: y^T + projection =================
            with (
                tc.tile_pool(name="stage_e", bufs=3) as se,
                tc.tile_pool(name="yT_pool", bufs=1) as ytp,
                tc.tile_pool(name="ps_yt", bufs=2, space="PSUM") as psyt,
                tc.tile_pool(name="ps_o", bufs=2, space="PSUM") as pso,
            ):
                yT = ytp.tile([P, PAIRS, T], bf16)
                for nt in range(NT):
                    ps = psyt.tile([P, 512], bf16, tag="yt")
                    for prr in range(PAIRS):
                        nc.tensor.transpose(
                            ps[:, prr * P : (prr + 1) * P],
                            y_nat[:, nt, prr * P : (prr + 1) * P],
                            ident,
                        )
                    nc.vector.tensor_copy(
                        yT[:, :, nt * P : (nt + 1) * P],
                        ps.rearrange("p (a b) -> p a b", b=P),
                    )
                for nt in range(NT):
                    o_sb = se.tile([P, D], fp32, tag="osb")
                    for ec in range(2):
                        o_ps = pso.tile([P, 512], fp32, tag="o")
                        for prr in range(PAIRS):
                            nc.tensor.matmul(
                                o_ps,
                                yT[:, prr, nt * P : (nt + 1) * P],
                                wpT[:, prr, ec * 512 : (ec + 1) * 512],
                                start=(prr == 0), stop=(prr == PAIRS - 1),
                            )
                        nc.vector.tensor_copy(
                            o_sb[:, ec * 512 : (ec + 1) * 512], o_ps
                        )
                    nc.sync.dma_start(out3[nt], o_sb)

    nc.compile()
    return nc


def _get_program():
    if "nc" not in _CACHE:
        _CACHE["nc"] = _build_program()
    return _CACHE["nc"]


def make_in_maps(x, Wq, Wk, Wv, Wproj, q_gain):
    cos, sin = _rope_tables()
    in_maps = []
    for c in range(8):
        b, hh = c // 2, c % 2
        in_maps.append(
            {
                "x": np.ascontiguousarray(x[b]),
                "wq": np.ascontiguousarray(Wq[hh * 512 : (hh + 1) * 512]),
                "wk": np.ascontiguousarray(Wk[hh * 128 : (hh + 1) * 128]),
                "wv": np.ascontiguousarray(Wv[hh * 128 : (hh + 1) * 128]),
                "wp": np.ascontiguousarray(Wproj[:, hh * 512 : (hh + 1) * 512]),
                "cos": cos,
                "sin": sin,
                "gain": np.ascontiguousarray(
                    np.broadcast_to(q_gain[hh * 8 : (hh + 1) * 8], (P, HL))
                ),
            }
        )
    return in_maps


def kernel(x, Wq, Wk, Wv, Wproj, q_gain):
    from concourse import bass_utils

    x = np.asarray(x, dtype=np.float32)
    Wq = np.asarray(Wq, dtype=np.float32)
    Wk = np.asarray(Wk, dtype=np.float32)
    Wv = np.asarray(Wv, dtype=np.float32)
    Wproj = np.asarray(Wproj, dtype=np.float32)
    q_gain = np.asarray(q_gain, dtype=np.float32)

    nc = _get_program()
    in_maps = make_in_maps(x, Wq, Wk, Wv, Wproj, q_gain)
    res = bass_utils.run_bass_kernel_spmd(
        nc, in_maps, core_ids=list(range(8)), trace=False
    )
    out = np.empty((B, T, D), dtype=np.float32)
    for b in range(B):
        out[b] = res.results[2 * b]["out"] + res.results[2 * b + 1]["out"]
    return out


# revision 23
# speedup vs baseline: 1.0157x; 1.0157x over previous
"""Causal GQA self-attention (B=4,T=2048,D=1024,H=16,HKV=4) on 8 trn2 cores.

Sharding: core c -> (batch b=c//2, head-half hh=c%2). Each core computes
8 query heads / 2 KV heads for one batch, plus the output projection
restricted to its 512 y-channels (full e). Host sums the two partial
projections per batch.

Pipeline per core (bf16 matmuls, fp32 accumulate), strip-pipelined so
QKV production, attention (ACT-bound exp), and the output projection
overlap:
  weights^T via PE transpose -> per t-tile: x^T, Q/K/V, RMSnorm
  (rsqrt = exp(-0.5*ln(ms+eps)), same ACT table set as softmax exp)
  + RoPE -> q^T, k^T (k replicated to both partition halves so the two
  K=64 score matmuls of a head pair row-pack the PE array) -> per
  512-wide query strip: scores^T per 128-key block, wide exp on ACT
  (no max subtraction; scores are bounded), triangular mask on the
  diagonal block only, AV with ones-augmented V (denominator = column
  64), per-partition normalize -> y^T -> projection -> DMA out.
"""

import numpy as np

B, T, D = 4, 2048, 1024
H, HKV, HD = 16, 4, 64
P = 128
NT = T // P          # 16 t-tiles
DC = D // P          # 8 contraction chunks
HL = H // 2          # 8 local q heads
PAIRS = HL // 2      # 4 head pairs
NS = 4               # query strips of 512
ROPE_BASE = 10000.0
EPS = 1.1920928955078125e-07
SCALE = 1.0 / 8.0    # 1/sqrt(HD)

_CACHE = {}


def _rope_tables():
    inv = (1.0 / (ROPE_BASE ** (np.arange(0, HD, 2, dtype=np.float32) / HD))).astype(
        np.float32
    )
    t = np.arange(T, dtype=np.float32)
    f = np.outer(t, inv).astype(np.float32)
    return np.cos(f).astype(np.float32), np.sin(f).astype(np.float32)


def _build_program():
    import concourse.mybir as mybir
    import concourse.tile as tile
    from concourse import bacc
    from concourse.masks import make_identity, make_upper_triangular

    fp32 = mybir.dt.float32
    bf16 = mybir.dt.bfloat16
    AX = mybir.AxisListType.X
    MUL = mybir.AluOpType.mult
    ADD = mybir.AluOpType.add
    SUB = mybir.AluOpType.subtract
    EXP = mybir.ActivationFunctionType.Exp
    SQRT = mybir.ActivationFunctionType.Sqrt

    nc = bacc.Bacc("TRN2", target_bir_lowering=False, debug=False)

    x_d = nc.dram_tensor("x", [T, D], fp32, kind="ExternalInput").ap()
    wq_d = nc.dram_tensor("wq", [HL * HD, D], fp32, kind="ExternalInput").ap()
    wk_d = nc.dram_tensor("wk", [2 * HD, D], fp32, kind="ExternalInput").ap()
    wv_d = nc.dram_tensor("wv", [2 * HD, D], fp32, kind="ExternalInput").ap()
    wp_d = nc.dram_tensor("wp", [D, HL * HD], fp32, kind="ExternalInput").ap()
    cos_d = nc.dram_tensor("cos", [T, HD // 2], fp32, kind="ExternalInput").ap()
    sin_d = nc.dram_tensor("sin", [T, HD // 2], fp32, kind="ExternalInput").ap()
    gain_d = nc.dram_tensor("gain", [P, HL], fp32, kind="ExternalInput").ap()
    out_d = nc.dram_tensor("out", [T, D], fp32, kind="ExternalOutput").ap()

    x3 = x_d.rearrange("(n p) d -> n p d", p=P)
    out3 = out_d.rearrange("(n p) d -> n p d", p=P)

    with tile.TileContext(nc) as tc:
        with (
            tc.tile_pool(name="persist", bufs=1) as persist,
            tc.tile_pool(name="p_pool", bufs=2) as pp,
            tc.tile_pool(name="yT_pool", bufs=2) as ytp,
            tc.tile_pool(name="stage_e", bufs=2) as se,
            tc.tile_pool(name="small", bufs=8) as sm,
            tc.tile_pool(name="ps_att", bufs=2, space="PSUM") as psat,
            tc.tile_pool(name="ps_y", bufs=2, space="PSUM") as psy,
        ):
            # ---- constants ----
            ident = persist.tile([P, P], bf16)
            make_identity(nc, ident)
            identf = persist.tile([P, P], fp32)
            make_identity(nc, identf)
            dmask = persist.tile([P, P], bf16)
            make_upper_triangular(nc, dmask, val=1.0, diag=True)
            cos_sb = persist.tile([P, NT, HD // 2], bf16)
            sin_sb = persist.tile([P, NT, HD // 2], bf16)
            nc.gpsimd.dma_start(cos_sb, cos_d.rearrange("(n p) c -> p n c", p=P))
            nc.gpsimd.dma_start(sin_sb, sin_d.rearrange("(n p) c -> p n c", p=P))
            gain_sb = persist.tile([P, HL], fp32)
            nc.sync.dma_start(gain_sb, gain_d)
            eps_sb = persist.tile([P, 1], fp32)
            nc.vector.memset(eps_sb, EPS)

            # ---- persistent activations / weights ----
            qT = persist.tile([P, PAIRS, T], bf16)   # [2-head dims, pair, t]
            kT2 = persist.tile([P, 2, T], bf16)      # kv heads, replicated halves
            v_sb = persist.tile([P, NT, 2 * (HD + 1)], bf16)  # ones-augmented
            y_nat = persist.tile([P, NT, HL * HD], bf16)
            wpT = persist.tile([P, PAIRS, D], bf16)
            wqT = persist.tile([P, DC, HL * HD], bf16)
            wkT = persist.tile([P, DC, 2 * HD], bf16)
            wvT = persist.tile([P, DC, 2 * HD], bf16)

            v4 = v_sb.rearrange("p n (h x) -> p n h x", h=2)
            nc.gpsimd.memset(v4[:, :, :, HD : HD + 1], 1.0)

            def transpose_in(src_f32, n_chunk, dst, dst_col0, ncols, pool):
                # src [P, n_chunk*128] f32 -> dst[:, c, col0:+ncols] bf16
                for g0 in range(0, n_chunk, 4):
                    gn = min(4, n_chunk - g0)
                    ps = pool.tile([P, 512], fp32, tag="m")
                    for gi in range(gn):
                        c = g0 + gi
                        nc.tensor.transpose(
                            ps[:, gi * P : (gi + 1) * P],
                            src_f32[:, c * P : (c + 1) * P],
                            identf,
                        )
                    nc.vector.tensor_copy(
                        dst[:, g0 : g0 + gn, dst_col0 : dst_col0 + ncols],
                        ps.rearrange("p (a b) -> p a b", b=P)[:, :gn, :ncols],
                    )

            # ================= weights =================
            with (
                tc.tile_pool(name="stage_w", bufs=3) as stw,
                tc.tile_pool(name="ps_w", bufs=2, space="PSUM") as psw,
            ):
                for rt in range(4):  # wq rows: e = rt*128 + p
                    wc = stw.tile([P, D], fp32, tag="wc")
                    nc.sync.dma_start(
                        wc, wq_d.rearrange("(r p) d -> r p d", p=P)[rt]
                    )
                    transpose_in(wc, DC, wqT, rt * P, P, psw)
                wc = stw.tile([P, D], fp32, tag="wc")
                nc.sync.dma_start(wc, wk_d)
                transpose_in(wc, DC, wkT, 0, P, psw)
                wc = stw.tile([P, D], fp32, tag="wc")
                nc.sync.dma_start(wc, wv_d)
                transpose_in(wc, DC, wvT, 0, P, psw)

            # ============ phase C: per-tile x^T, QKV, RMSnorm + RoPE, q^T/k^T.
            # All ACT Sqrts are emitted before any strip Exp (per-engine
            # order follows program order), so only 2 table-set loads occur.
            with (
                tc.tile_pool(name="stage_c", bufs=2) as sc_st,
                tc.tile_pool(name="xf_pool", bufs=4) as xfp,
                tc.tile_pool(name="ps_cqkv", bufs=1, space="PSUM") as psqkv,
                tc.tile_pool(name="ps_cm", bufs=1, space="PSUM") as pscm,
            ):

                def norm_rope(nt, src_ps, nh, gain):
                    # src_ps [P, nh*HD] fp32 psum -> roped+normed bf16
                    sb = sc_st.tile([P, nh * HD], bf16, tag=f"sb{nh}")
                    nc.vector.tensor_copy(sb, src_ps)
                    s3 = sb.rearrange("p (h x) -> p h x", h=nh)
                    sq = sc_st.tile([P, nh * HD], bf16, tag=f"sq{nh}")
                    nc.vector.tensor_tensor(sq, sb, sb, MUL)
                    ss = sc_st.tile([P, nh], fp32, tag=f"ss{nh}")
                    nc.vector.reduce_sum(
                        ss, sq.rearrange("p (h x) -> p h x", h=nh), axis=AX
                    )
                    rms = sc_st.tile([P, nh], fp32, tag=f"rm{nh}")
                    nc.scalar.activation(
                        rms, ss, SQRT, bias=eps_sb[:, 0:1], scale=1.0 / HD
                    )
                    inv = sc_st.tile([P, nh], fp32, tag=f"iv{nh}")
                    nc.vector.reciprocal(inv, rms)
                    if gain is not None:
                        nc.vector.tensor_tensor(inv, inv, gain, MUL)
                    h2 = HD // 2
                    x1 = s3[:, :, 0:h2]
                    x2 = s3[:, :, h2:HD]
                    cb = cos_sb[:, nt : nt + 1, :].to_broadcast([P, nh, h2])
                    sbr = sin_sb[:, nt : nt + 1, :].to_broadcast([P, nh, h2])
                    r = sc_st.tile([P, nh * HD], bf16, tag=f"r{nh}")
                    r3 = r.rearrange("p (h x) -> p h x", h=nh)
                    tmp = sc_st.tile([P, nh * (HD // 2)], bf16, tag=f"t{nh}")
                    t3 = tmp.rearrange("p (h x) -> p h x", h=nh)
                    nc.vector.tensor_tensor(r3[:, :, 0:h2], x1, cb, MUL)
                    nc.vector.tensor_tensor(t3, x2, sbr, MUL)
                    nc.vector.tensor_tensor(
                        r3[:, :, 0:h2], r3[:, :, 0:h2], t3, ADD
                    )
                    nc.vector.tensor_tensor(r3[:, :, h2:HD], x2, cb, MUL)
                    nc.vector.tensor_tensor(t3, x1, sbr, MUL)
                    nc.vector.tensor_tensor(
                        r3[:, :, h2:HD], r3[:, :, h2:HD], t3, SUB
                    )
                    ivb = inv[:, :, None].to_broadcast([P, nh, HD])
                    nc.vector.tensor_tensor(r3, r3, ivb, MUL)
                    return r

                for nt in range(NT):
                    xf = xfp.tile([P, D], fp32, tag="xf")
                    nc.sync.dma_start(xf, x3[nt])
                    xTt = sc_st.tile([P, DC, P], bf16, tag="xT")
                    transpose_in(xf, DC, xTt, 0, P, pscm)

                    q_ps = psqkv.tile([P, HL * HD], fp32, tag="qkv")
                    for dc in range(DC):
                        nc.tensor.matmul(
                            q_ps, xTt[:, dc, :], wqT[:, dc, :],
                            start=(dc == 0), stop=(dc == DC - 1),
                        )
                    k_full = psqkv.tile([P, HL * HD], fp32, tag="qkv")
                    k_ps = k_full[:, 0 : 2 * HD]
                    for dc in range(DC):
                        nc.tensor.matmul(
                            k_ps, xTt[:, dc, :], wkT[:, dc, :],
                            start=(dc == 0), stop=(dc == DC - 1),
                        )
                    kr = norm_rope(nt, k_ps, 2, None)
                    v_full = psqkv.tile([P, HL * HD], fp32, tag="qkv")
                    v_ps = v_full[:, 0 : 2 * HD]
                    for dc in range(DC):
                        nc.tensor.matmul(
                            v_ps, xTt[:, dc, :], wvT[:, dc, :],
                            start=(dc == 0), stop=(dc == DC - 1),
                        )
                    nc.vector.tensor_copy(
                        v4[:, nt, :, 0:HD],
                        v_ps.rearrange("p (h x) -> p h x", h=2),
                    )
                    qr = norm_rope(nt, q_ps, HL, gain_sb)

                    # q^T: 4 pair transposes
                    ps = pscm.tile([P, 512], bf16, tag="m")
                    for pr in range(PAIRS):
                        nc.tensor.transpose(
                            ps[:, pr * P : (pr + 1) * P],
                            qr[:, pr * P : (pr + 1) * P],
                            ident,
                        )
                    nc.vector.tensor_copy(
                        qT[:, :, nt * P : (nt + 1) * P],
                        ps.rearrange("p (a b) -> p a b", b=P),
                    )
                    # k^T replicated to both partition halves
                    kps = pscm.tile([P, 2, P], bf16, tag="m")
                    for kv in range(2):
                        for rep in range(2):
                            nc.tensor.transpose(
                                kps[rep * 64 : (rep + 1) * 64, kv, :],
                                kr[:, kv * HD : (kv + 1) * HD],
                                ident,
                                tile_position=(0, rep * 64),
                            )
                    nc.vector.tensor_copy(
                        kT2[:, :, nt * P : (nt + 1) * P], kps
                    )

            # wp^T transposes: only needed by the projection; emitted after
            # phase C so they fill early-attention PE gaps.
            with (
                tc.tile_pool(name="stage_w2", bufs=2) as stw2,
                tc.tile_pool(name="ps_w2", bufs=1, space="PSUM") as psw2,
            ):
                for rt in range(DC):  # wp rows: e = rt*128 + p
                    wc = stw2.tile([P, HL * HD], fp32, tag="wpc")
                    nc.sync.dma_start(
                        wc, wp_d.rearrange("(r p) d -> r p d", p=P)[rt]
                    )
                    transpose_in(wc, PAIRS, wpT, rt * P, P, psw2)

            # ========= attention + projection, per 512-wide query strip =========
            # ps_e opens after phase C's psum pools close: the projection
            # reuses those banks (serializes only against tail-of-phase-C).
            with tc.tile_pool(name="ps_e", bufs=1, space="PSUM") as pse:
              for s in range(NS):
                tq0 = s * 512
                for pr in range(PAIRS):
                    kv = pr // 2
                    p_tiles = {}
                    for tkb in range(4 * s + 4):
                        m = tkb - 4 * s  # >=0 only for diagonal-strip blocks
                        c0 = max(m, 0) * P   # first causally-valid strip column
                        pt = pp.tile([P, 2, 512], bf16, tag=f"p{tkb}")
                        p_tiles[tkb] = pt
                        sc = psat.tile([P, 2, 512], fp32, tag="sc")
                        for h01 in range(2):
                            hp = h01 * 64
                            nc.tensor.matmul(
                                sc[:, h01, c0:512],
                                kT2[hp : hp + 64, kv, tkb * P : (tkb + 1) * P],
                                qT[hp : hp + 64, pr, tq0 + c0 : tq0 + 512],
                                start=True, stop=True,
                            )
                        nc.scalar.activation(
                            pt[:, :, c0:512], sc[:, :, c0:512], EXP, scale=SCALE
                        )
                        if m >= 0:
                            # triangular mask on the diagonal 128-block
                            # (gpsimd: idle engine, keeps DVE seq free)
                            dm = dmask[:, None, :].to_broadcast([P, 2, P])
                            nc.gpsimd.tensor_tensor(
                                pt[:, :, c0 : c0 + P],
                                pt[:, :, c0 : c0 + P], dm, MUL,
                            )
                    for tqi in range(4 * s, 4 * s + 4):
                        co = (tqi - 4 * s) * P
                        for h01 in range(2):
                            y_ps = psy.tile([P, HD + 1], fp32, tag="y")
                            for tkb in range(tqi + 1):
                                nc.tensor.matmul(
                                    y_ps,
                                    p_tiles[tkb][:, h01, co : co + P],
                                    v_sb[:, tkb,
                                         kv * (HD + 1) : (kv + 1) * (HD + 1)],
                                    start=(tkb == 0), stop=(tkb == tqi),
                                )
                            rcp = sm.tile([P, 1], fp32, tag="rcp")
                            nc.vector.reciprocal(rcp, y_ps[:, HD : HD + 1])
                            h = 2 * pr + h01
                            nc.vector.tensor_tensor(
                                y_nat[:, tqi, h * HD : (h + 1) * HD],
                                y_ps[:, 0:HD],
                                rcp[:, 0:1].to_broadcast([P, HD]),
                                MUL,
                            )

                # ---- y^T + projection for this strip ----
                yTs = ytp.tile([P, PAIRS, 512], bf16, tag="yT")
                for j in range(4):
                    nt = 4 * s + j
                    ps = pse.tile([P, 512], bf16, tag="yt")
                    for prr in range(PAIRS):
                        nc.tensor.transpose(
                            ps[:, prr * P : (prr + 1) * P],
                            y_nat[:, nt, prr * P : (prr + 1) * P],
                            ident,
                        )
                    nc.vector.tensor_copy(
                        yTs[:, :, j * P : (j + 1) * P],
                        ps.rearrange("p (a b) -> p a b", b=P),
                    )
                for j in range(4):
                    nt = 4 * s + j
                    o_sb = se.tile([P, D], fp32, tag="osb")
                    for ec in range(2):
                        o_ps = pse.tile([P, 512], fp32, tag="o")
                        for prr in range(PAIRS):
                            nc.tensor.matmul(
                                o_ps,
                                yTs[:, prr, j * P : (j + 1) * P],
                                wpT[:, prr, ec * 512 : (ec + 1) * 512],
                                start=(prr == 0), stop=(prr == PAIRS - 1),
                            )
                        nc.vector.tensor_copy(
                            o_sb[:, ec * 512 : (ec + 1) * 512], o_ps
                        )
                    nc.sync.dma_start(out3[nt], o_sb)

    nc.compile()
    return nc


def _get_program():
    if "nc" not in _CACHE:
        _CACHE["nc"] = _build_program()
    return _CACHE["nc"]


def make_in_maps(x, Wq, Wk, Wv, Wproj, q_gain):
    cos, sin = _rope_tables()
    in_maps = []
    for c in range(8):
        b, hh = c // 2, c % 2
        in_maps.append(
            {
                "x": np.ascontiguousarray(x[b]),
                "wq": np.ascontiguousarray(Wq[hh * 512 : (hh + 1) * 512]),
                "wk": np.ascontiguousarray(Wk[hh * 128 : (hh + 1) * 128]),
                "wv": np.ascontiguousarray(Wv[hh * 128 : (hh + 1) * 128]),
                "wp": np.ascontiguousarray(Wproj[:, hh * 512 : (hh + 1) * 512]),
                "cos": cos,
                "sin": sin,
                "gain": np.ascontiguousarray(
                    np.broadcast_to(q_gain[hh * 8 : (hh + 1) * 8], (P, HL))
                ),
            }
        )
    return in_maps


def kernel(x, Wq, Wk, Wv, Wproj, q_gain):
    from concourse import bass_utils

    x = np.asarray(x, dtype=np.float32)
    Wq = np.asarray(Wq, dtype=np.float32)
    Wk = np.asarray(Wk, dtype=np.float32)
    Wv = np.asarray(Wv, dtype=np.float32)
    Wproj = np.asarray(Wproj, dtype=np.float32)
    q_gain = np.asarray(q_gain, dtype=np.float32)

    nc = _get_program()
    in_maps = make_in_maps(x, Wq, Wk, Wv, Wproj, q_gain)
    res = bass_utils.run_bass_kernel_spmd(
        nc, in_maps, core_ids=list(range(8)), trace=False
    )
    out = np.empty((B, T, D), dtype=np.float32)
    for b in range(B):
        out[b] = res.results[2 * b]["out"] + res.results[2 * b + 1]["out"]
    return out


# revision 32
# speedup vs baseline: 252.8116x; 248.9075x over previous
"""Causal GQA self-attention (B=4,T=2048,D=1024,H=16,HKV=4) on 8 trn2 cores.

Sharding: core c -> (batch b=c//2, head-half hh=c%2). Each core computes
8 query heads / 2 KV heads for one batch, plus the output projection
restricted to its 512 y-channels (full e). Host sums the two partial
projections per batch.

Pipeline per core (bf16 matmuls, fp32 accumulate), software-pipelined so
QKV production, attention (ScalarE-bound exp), and the projection overlap:
  weights^T via PE transpose -> per t-tile: x^T, fused QKV (k,v share one
  matmul group), RMSnorm (ACT Sqrt + DVE reciprocal; all Sqrts precede
  all Exps in the ACT stream so only two table-set loads occur) + RoPE ->
  q^T, k^T (k replicated to both partition halves so the two K=64 score
  matmuls of a head pair row-pack the PE array via tile_position
  auto-derivation) -> per 512-wide query strip: scores^T per 128-key
  block, wide exp on ACT (no max subtraction; post-norm scores are
  bounded by ~12), triangular mask on the diagonal block only (gpsimd),
  AV with ones-augmented V (softmax denominator = column 64),
  per-partition normalize -> y^T -> projection (deferred past phase C so
  its PSUM banks are reused) -> DMA out.
"""

import numpy as np

B, T, D = 4, 2048, 1024
H, HKV, HD = 16, 4, 64
P = 128
NT = T // P          # 16 t-tiles
DC = D // P          # 8 contraction chunks
HL = H // 2          # 8 local q heads
PAIRS = HL // 2      # 4 head pairs
NS = 4               # query strips of 512
ROPE_BASE = 10000.0
EPS = 1.1920928955078125e-07
SCALE = 1.0 / 8.0    # 1/sqrt(HD)

_CACHE = {}


def _rope_tables():
    inv = (1.0 / (ROPE_BASE ** (np.arange(0, HD, 2, dtype=np.float32) / HD))).astype(
        np.float32
    )
    t = np.arange(T, dtype=np.float32)
    f = np.outer(t, inv).astype(np.float32)
    return np.cos(f).astype(np.float32), np.sin(f).astype(np.float32)


def _build_program():
    import concourse.mybir as mybir
    import concourse.tile as tile
    from concourse import bacc
    from concourse.masks import make_identity, make_upper_triangular

    fp32 = mybir.dt.float32
    bf16 = mybir.dt.bfloat16
    AX = mybir.AxisListType.X
    MUL = mybir.AluOpType.mult
    ADD = mybir.AluOpType.add
    SUB = mybir.AluOpType.subtract
    EXP = mybir.ActivationFunctionType.Exp
    SQRT = mybir.ActivationFunctionType.Sqrt

    nc = bacc.Bacc("TRN2", target_bir_lowering=False, debug=False)

    x_d = nc.dram_tensor("x", [T, D], fp32, kind="ExternalInput").ap()
    wq_d = nc.dram_tensor("wq", [HL * HD, D], fp32, kind="ExternalInput").ap()
    wk_d = nc.dram_tensor("wk", [2 * HD, D], fp32, kind="ExternalInput").ap()
    wv_d = nc.dram_tensor("wv", [2 * HD, D], fp32, kind="ExternalInput").ap()
    wp_d = nc.dram_tensor("wp", [D, HL * HD], fp32, kind="ExternalInput").ap()
    cos_d = nc.dram_tensor("cos", [T, HD // 2], fp32, kind="ExternalInput").ap()
    sin_d = nc.dram_tensor("sin", [T, HD // 2], fp32, kind="ExternalInput").ap()
    gain_d = nc.dram_tensor("gain", [P, HL], fp32, kind="ExternalInput").ap()
    out_d = nc.dram_tensor("out", [T, D], fp32, kind="ExternalOutput").ap()

    x3 = x_d.rearrange("(n p) d -> n p d", p=P)
    out3 = out_d.rearrange("(n p) d -> n p d", p=P)

    with tile.TileContext(nc) as tc:
        with (
            tc.tile_pool(name="persist", bufs=1) as persist,
            tc.tile_pool(name="p_pool", bufs=2) as pp,
            tc.tile_pool(name="yT_pool", bufs=2) as ytp,
            tc.tile_pool(name="stage_e", bufs=2) as se,
            tc.tile_pool(name="small", bufs=8) as sm,
            tc.tile_pool(name="ps_att", bufs=2, space="PSUM") as psat,
            tc.tile_pool(name="ps_y", bufs=2, space="PSUM") as psy,
        ):
            # ---- constants ----
            ident = persist.tile([P, P], bf16)
            make_identity(nc, ident)
            identf = persist.tile([P, P], fp32)
            make_identity(nc, identf)
            dmask = persist.tile([P, P], bf16)
            make_upper_triangular(nc, dmask, val=1.0, diag=True)
            cos_sb = persist.tile([P, NT, HD // 2], bf16)
            sin_sb = persist.tile([P, NT, HD // 2], bf16)
            nc.gpsimd.dma_start(cos_sb, cos_d.rearrange("(n p) c -> p n c", p=P))
            nc.gpsimd.dma_start(sin_sb, sin_d.rearrange("(n p) c -> p n c", p=P))
            gain_sb = persist.tile([P, HL], fp32)
            nc.sync.dma_start(gain_sb, gain_d)
            eps_sb = persist.tile([P, 1], fp32)
            nc.vector.memset(eps_sb, EPS)

            # ---- persistent activations / weights ----
            qT = persist.tile([P, PAIRS, T], bf16)   # [2-head dims, pair, t]
            kT2 = persist.tile([P, 2, T], bf16)      # kv heads, replicated halves
            v_sb = persist.tile([P, NT, 2 * (HD + 1)], bf16)  # ones-augmented
            y_nat = persist.tile([P, NT, HL * HD], bf16)
            wpT = persist.tile([P, PAIRS, D], bf16)
            wqT = persist.tile([P, DC, HL * HD], bf16)
            wkvT = persist.tile([P, DC, 4 * HD], bf16)

            v4 = v_sb.rearrange("p n (h x) -> p n h x", h=2)
            nc.gpsimd.memset(v4[:, :, :, HD : HD + 1], 1.0)

            def transpose_in(src_f32, n_chunk, dst, dst_col0, ncols, pool,
                             copy_eng=None):
                # src [P, n_chunk*128] f32 -> dst[:, c, col0:+ncols] bf16
                for g0 in range(0, n_chunk, 4):
                    gn = min(4, n_chunk - g0)
                    ps = pool.tile([P, 512], fp32, tag="m")
                    for gi in range(gn):
                        c = g0 + gi
                        nc.tensor.transpose(
                            ps[:, gi * P : (gi + 1) * P],
                            src_f32[:, c * P : (c + 1) * P],
                            identf,
                        )
                    if copy_eng == "scalar":
                        nc.scalar.copy(
                            dst[:, g0 : g0 + gn, dst_col0 : dst_col0 + ncols],
                            ps.rearrange("p (a b) -> p a b", b=P)[:, :gn, :ncols],
                        )
                    else:
                        nc.vector.tensor_copy(
                            dst[:, g0 : g0 + gn, dst_col0 : dst_col0 + ncols],
                            ps.rearrange("p (a b) -> p a b", b=P)[:, :gn, :ncols],
                        )

            # ================= weights =================
            with (
                tc.tile_pool(name="stage_w", bufs=3) as stw,
                tc.tile_pool(name="ps_w", bufs=2, space="PSUM") as psw,
            ):
                for rt in range(4):  # wq rows: e = rt*128 + p
                    wc = stw.tile([P, D], fp32, tag="wc")
                    nc.sync.dma_start(
                        wc, wq_d.rearrange("(r p) d -> r p d", p=P)[rt]
                    )
                    transpose_in(wc, DC, wqT, rt * P, P, psw)
                wc = stw.tile([P, D], fp32, tag="wc")
                nc.sync.dma_start(wc, wk_d)
                transpose_in(wc, DC, wkvT, 0, P, psw)
                wc = stw.tile([P, D], fp32, tag="wc")
                nc.sync.dma_start(wc, wv_d)
                transpose_in(wc, DC, wkvT, 2 * HD, P, psw)

            # ===== phase C tiles interleaved with attention strips =====
            # Emission (and so each engine's static order) alternates four
            # QKV tiles with the strip they complete, so strip g's scores/
            # exp start as soon as q^T/k^T tiles 0..4g+3 exist. Projections
            # are deferred past phase C so their PSUM can reuse its banks.
            with (
                tc.tile_pool(name="stage_c", bufs=2) as sc_st,
                tc.tile_pool(name="xf_pool", bufs=4) as xfp,
                tc.tile_pool(name="ps_cqkv", bufs=1, space="PSUM") as psqkv,
                tc.tile_pool(name="ps_cm", bufs=1, space="PSUM") as pscm,
            ):

                def norm_rope(nt, src_ps, nh, gain):
                    # src_ps [P, nh*HD] fp32 psum -> roped+normed bf16
                    sb = sc_st.tile([P, nh * HD], bf16, tag=f"sb{nh}")
                    nc.vector.tensor_copy(sb, src_ps)
                    s3 = sb.rearrange("p (h x) -> p h x", h=nh)
                    sq = sc_st.tile([P, nh * HD], bf16, tag=f"sq{nh}")
                    nc.vector.tensor_tensor(sq, sb, sb, MUL)
                    ss = sc_st.tile([P, nh], fp32, tag=f"ss{nh}")
                    nc.vector.reduce_sum(
                        ss, sq.rearrange("p (h x) -> p h x", h=nh), axis=AX
                    )
                    rms = sc_st.tile([P, nh], fp32, tag=f"rm{nh}")
                    nc.scalar.activation(
                        rms, ss, SQRT, bias=eps_sb[:, 0:1], scale=1.0 / HD
                    )
                    inv = sc_st.tile([P, nh], fp32, tag=f"iv{nh}")
                    nc.vector.reciprocal(inv, rms)
                    if gain is not None:
                        nc.vector.tensor_tensor(inv, inv, gain, MUL)
                    h2 = HD // 2
                    x1 = s3[:, :, 0:h2]
                    x2 = s3[:, :, h2:HD]
                    cb = cos_sb[:, nt : nt + 1, :].to_broadcast([P, nh, h2])
                    sbr = sin_sb[:, nt : nt + 1, :].to_broadcast([P, nh, h2])
                    r = sc_st.tile([P, nh * HD], bf16, tag=f"r{nh}")
                    r3 = r.rearrange("p (h x) -> p h x", h=nh)
                    tmp = sc_st.tile([P, nh * (HD // 2)], bf16, tag=f"t{nh}")
                    t3 = tmp.rearrange("p (h x) -> p h x", h=nh)
                    nc.vector.tensor_tensor(r3[:, :, 0:h2], x1, cb, MUL)
                    nc.vector.tensor_tensor(t3, x2, sbr, MUL)
                    nc.vector.tensor_tensor(
                        r3[:, :, 0:h2], r3[:, :, 0:h2], t3, ADD
                    )
                    nc.vector.tensor_tensor(r3[:, :, h2:HD], x2, cb, MUL)
                    nc.vector.tensor_tensor(t3, x1, sbr, MUL)
                    nc.vector.tensor_tensor(
                        r3[:, :, h2:HD], r3[:, :, h2:HD], t3, SUB
                    )
                    ivb = inv[:, :, None].to_broadcast([P, nh, HD])
                    nc.vector.tensor_tensor(r3, r3, ivb, MUL)
                    return r

                def tile_c(nt):
                    xf = xfp.tile([P, D], fp32, tag="xf")
                    nc.sync.dma_start(xf, x3[nt])
                    xTt = sc_st.tile([P, DC, P], bf16, tag="xT")
                    transpose_in(xf, DC, xTt, 0, P, pscm)

                    q_ps = psqkv.tile([P, HL * HD], fp32, tag="qkv")
                    for dc in range(DC):
                        nc.tensor.matmul(
                            q_ps, xTt[:, dc, :], wqT[:, dc, :],
                            start=(dc == 0), stop=(dc == DC - 1),
                        )
                    kv_full = psqkv.tile([P, HL * HD], fp32, tag="qkv")
                    for dc in range(DC):
                        nc.tensor.matmul(
                            kv_full[:, 0 : 4 * HD], xTt[:, dc, :], wkvT[:, dc, :],
                            start=(dc == 0), stop=(dc == DC - 1),
                        )
                    k_ps = kv_full[:, 0 : 2 * HD]
                    v_ps = kv_full[:, 2 * HD : 4 * HD]
                    nc.scalar.copy(
                        v4[:, nt, :, 0:HD],
                        v_ps.rearrange("p (h x) -> p h x", h=2),
                    )
                    kr = norm_rope(nt, k_ps, 2, None)
                    qr = norm_rope(nt, q_ps, HL, gain_sb)

                    # q^T: 4 pair transposes
                    ps = pscm.tile([P, 512], bf16, tag="m")
                    for pr in range(PAIRS):
                        nc.tensor.transpose(
                            ps[:, pr * P : (pr + 1) * P],
                            qr[:, pr * P : (pr + 1) * P],
                            ident,
                        )
                    nc.vector.tensor_copy(
                        qT[:, :, nt * P : (nt + 1) * P],
                        ps.rearrange("p (a b) -> p a b", b=P),
                    )
                    # k^T replicated to both partition halves
                    kps = pscm.tile([P, 2, P], bf16, tag="m")
                    for kv in range(2):
                        for rep in range(2):
                            nc.tensor.transpose(
                                kps[rep * 64 : (rep + 1) * 64, kv, :],
                                kr[:, kv * HD : (kv + 1) * HD],
                                ident,
                                tile_position=(0, rep * 64),
                            )
                    nc.scalar.copy(
                        kT2[:, :, nt * P : (nt + 1) * P], kps
                    )

                def emit_scores(s, pr):
                    tq0 = s * 512
                    kv = pr // 2
                    p_tiles = {}
                    for tkb in range(4 * s + 4):
                        m = tkb - 4 * s  # >=0 only for diagonal-strip blocks
                        c0 = max(m, 0) * P   # first causally-valid strip col
                        pt = pp.tile([P, 2, 512], bf16, tag=f"p{tkb}")
                        p_tiles[tkb] = pt
                        sc = psat.tile([P, 2, 512], fp32, tag="sc")
                        for h01 in range(2):
                            hp = h01 * 64
                            nc.tensor.matmul(
                                sc[:, h01, c0:512],
                                kT2[hp : hp + 64, kv, tkb * P : (tkb + 1) * P],
                                qT[hp : hp + 64, pr, tq0 + c0 : tq0 + 512],
                                start=True, stop=True,
                            )
                        nc.scalar.activation(
                            pt[:, :, c0:512], sc[:, :, c0:512], EXP, scale=SCALE
                        )
                        if m >= 0:
                            # triangular mask on the diagonal 128-block
                            # (gpsimd: idle engine, keeps DVE free)
                            dm = dmask[:, None, :].to_broadcast([P, 2, P])
                            nc.gpsimd.tensor_tensor(
                                pt[:, :, c0 : c0 + P],
                                pt[:, :, c0 : c0 + P], dm, MUL,
                            )
                    return p_tiles

                def emit_av(s, pr, p_tiles):
                    kv = pr // 2
                    for tqi in range(4 * s, 4 * s + 4):
                        co = (tqi - 4 * s) * P
                        for h01 in range(2):
                            y_ps = psy.tile([P, HD + 1], fp32, tag="y")
                            for tkb in range(tqi + 1):
                                nc.tensor.matmul(
                                    y_ps,
                                    p_tiles[tkb][:, h01, co : co + P],
                                    v_sb[:, tkb,
                                         kv * (HD + 1) : (kv + 1) * (HD + 1)],
                                    start=(tkb == 0), stop=(tkb == tqi),
                                )
                            rcp = sm.tile([P, 1], fp32, tag="rcp")
                            nc.vector.reciprocal(rcp, y_ps[:, HD : HD + 1])
                            h = 2 * pr + h01
                            nc.vector.tensor_tensor(
                                y_nat[:, tqi, h * HD : (h + 1) * HD],
                                y_ps[:, 0:HD],
                                rcp[:, 0:1].to_broadcast([P, HD]),
                                MUL,
                            )

                import os as _os
                _interleave = _os.environ.get("K_EMIT", "flat") == "interleave"
                if _interleave:
                    # Global software pipeline: every AV(g,pr) (gated on its
                    # exps) is preceded in emission by independent PE work.
                    for nt in range(4):
                        tile_c(nt)
                    for g in range(3):
                        tiles = {0: emit_scores(g, 0)}
                        tiles[1] = emit_scores(g, 1)
                        emit_av(g, 0, tiles.pop(0))
                        tile_c(4 * g + 4)
                        tiles[2] = emit_scores(g, 2)
                        emit_av(g, 1, tiles.pop(1))
                        tile_c(4 * g + 5)
                        tiles[3] = emit_scores(g, 3)
                        emit_av(g, 2, tiles.pop(2))
                        tile_c(4 * g + 6)
                        tile_c(4 * g + 7)
                        emit_av(g, 3, tiles.pop(3))
                else:
                    for nt in range(NT):
                        tile_c(nt)
                    for g in range(3):
                        tiles_cur = emit_scores(g, 0)
                        for pr in range(PAIRS):
                            tiles_next = (
                                emit_scores(g, pr + 1) if pr + 1 < PAIRS else None
                            )
                            emit_av(g, pr, tiles_cur)
                            tiles_cur = tiles_next

            # wp^T transposes (feed only the projection)
            with (
                tc.tile_pool(name="stage_w2", bufs=2) as stw2,
                tc.tile_pool(name="ps_w2", bufs=1, space="PSUM") as psw2,
            ):
                for rt in range(DC):  # wp rows: e = rt*128 + p
                    wc = stw2.tile([P, HL * HD], fp32, tag="wpc")
                    nc.sync.dma_start(
                        wc, wp_d.rearrange("(r p) d -> r p d", p=P)[rt]
                    )
                    transpose_in(wc, PAIRS, wpT, rt * P, P, psw2)

            # ===== strip 3 interleaved with all projections =====
            with tc.tile_pool(name="ps_e", bufs=1, space="PSUM") as pse:

                def proj(s):
                    yTs = ytp.tile([P, PAIRS, 512], bf16, tag="yT")
                    for j in range(4):
                        nt = 4 * s + j
                        ps = pse.tile([P, 512], bf16, tag="yt")
                        for prr in range(PAIRS):
                            nc.tensor.transpose(
                                ps[:, prr * P : (prr + 1) * P],
                                y_nat[:, nt, prr * P : (prr + 1) * P],
                                ident,
                            )
                        nc.vector.tensor_copy(
                            yTs[:, :, j * P : (j + 1) * P],
                            ps.rearrange("p (a b) -> p a b", b=P),
                        )
                    for j in range(4):
                        nt = 4 * s + j
                        o_sb = se.tile([P, D], fp32, tag="osb")
                        for ec in range(2):
                            o_ps = pse.tile([P, 512], fp32, tag="o")
                            for prr in range(PAIRS):
                                nc.tensor.matmul(
                                    o_ps,
                                    yTs[:, prr, j * P : (j + 1) * P],
                                    wpT[:, prr, ec * 512 : (ec + 1) * 512],
                                    start=(prr == 0), stop=(prr == PAIRS - 1),
                                )
                            nc.vector.tensor_copy(
                                o_sb[:, ec * 512 : (ec + 1) * 512], o_ps
                            )
                        nc.sync.dma_start(out3[nt], o_sb)

                t30 = emit_scores(3, 0)
                t31 = emit_scores(3, 1)
                emit_av(3, 0, t30)
                proj(0)
                t32 = emit_scores(3, 2)
                emit_av(3, 1, t31)
                proj(1)
                t33 = emit_scores(3, 3)
                emit_av(3, 2, t32)
                proj(2)
                emit_av(3, 3, t33)
                proj(3)

    nc.compile()
    return nc


def _get_program():
    if "nc" not in _CACHE:
        _CACHE["nc"] = _build_program()
    return _CACHE["nc"]


def make_in_maps(x, Wq, Wk, Wv, Wproj, q_gain):
    cos, sin = _rope_tables()
    in_maps = []
    for c in range(8):
        b, hh = c // 2, c % 2
        in_maps.append(
            {
                "x": np.ascontiguousarray(x[b]),
                "wq": np.ascontiguousarray(Wq[hh * 512 : (hh + 1) * 512]),
                "wk": np.ascontiguousarray(Wk[hh * 128 : (hh + 1) * 128]),
                "wv": np.ascontiguousarray(Wv[hh * 128 : (hh + 1) * 128]),
                "wp": np.ascontiguousarray(Wproj[:, hh * 512 : (hh + 1) * 512]),
                "cos": cos,
                "sin": sin,
                "gain": np.ascontiguousarray(
                    np.broadcast_to(q_gain[hh * 8 : (hh + 1) * 8], (P, HL))
                ),
            }
        )
    return in_maps


def kernel(x, Wq, Wk, Wv, Wproj, q_gain):
    from concourse import bass_utils

    x = np.asarray(x, dtype=np.float32)
    Wq = np.asarray(Wq, dtype=np.float32)
    Wk = np.asarray(Wk, dtype=np.float32)
    Wv = np.asarray(Wv, dtype=np.float32)
    Wproj = np.asarray(Wproj, dtype=np.float32)
    q_gain = np.asarray(q_gain, dtype=np.float32)

    nc = _get_program()
    in_maps = make_in_maps(x, Wq, Wk, Wv, Wproj, q_gain)
    res = bass_utils.run_bass_kernel_spmd(
        nc, in_maps, core_ids=list(range(8)), trace=False
    )
    out = np.empty((B, T, D), dtype=np.float32)
    for b in range(B):
        out[b] = res.results[2 * b]["out"] + res.results[2 * b + 1]["out"]
    return out


# revision 35
# speedup vs baseline: 255.1745x; 1.0093x over previous
"""Causal GQA self-attention (B=4,T=2048,D=1024,H=16,HKV=4) on 8 trn2 cores.

Sharding: core c -> (batch b=c//2, head-half hh=c%2). Each core computes
8 query heads / 2 KV heads for one batch, plus the output projection
restricted to its 512 y-channels (full e). Host sums the two partial
projections per batch.

Pipeline per core (bf16 matmuls, fp32 accumulate), software-pipelined so
QKV production, attention (ScalarE-bound exp), and the projection overlap:
  weights^T via PE transpose -> per t-tile: x^T, fused QKV (k,v share one
  matmul group), RMSnorm (ACT Sqrt + DVE reciprocal; all Sqrts precede
  all Exps in the ACT stream so only two table-set loads occur) + RoPE ->
  q^T, k^T (k replicated to both partition halves so the two K=64 score
  matmuls of a head pair row-pack the PE array via tile_position
  auto-derivation) -> per 512-wide query strip: scores^T per 128-key
  block, wide exp on ACT (no max subtraction; post-norm scores are
  bounded by ~12), triangular mask on the diagonal block only (gpsimd),
  AV with ones-augmented V (softmax denominator = column 64),
  per-partition normalize -> y^T -> projection (deferred past phase C so
  its PSUM banks are reused) -> DMA out.
"""

import numpy as np

B, T, D = 4, 2048, 1024
H, HKV, HD = 16, 4, 64
P = 128
NT = T // P          # 16 t-tiles
DC = D // P          # 8 contraction chunks
HL = H // 2          # 8 local q heads
PAIRS = HL // 2      # 4 head pairs
NS = 4               # query strips of 512
ROPE_BASE = 10000.0
EPS = 1.1920928955078125e-07
SCALE = 1.0 / 8.0    # 1/sqrt(HD)

_CACHE = {}


def _rope_tables():
    inv = (1.0 / (ROPE_BASE ** (np.arange(0, HD, 2, dtype=np.float32) / HD))).astype(
        np.float32
    )
    t = np.arange(T, dtype=np.float32)
    f = np.outer(t, inv).astype(np.float32)
    return np.cos(f).astype(np.float32), np.sin(f).astype(np.float32)


def _build_program():
    import concourse.mybir as mybir
    import concourse.tile as tile
    from concourse import bacc
    from concourse.masks import make_identity, make_upper_triangular

    fp32 = mybir.dt.float32
    bf16 = mybir.dt.bfloat16
    AX = mybir.AxisListType.X
    MUL = mybir.AluOpType.mult
    ADD = mybir.AluOpType.add
    SUB = mybir.AluOpType.subtract
    EXP = mybir.ActivationFunctionType.Exp
    SQRT = mybir.ActivationFunctionType.Sqrt

    nc = bacc.Bacc("TRN2", target_bir_lowering=False, debug=False)

    x_d = nc.dram_tensor("x", [T, D], fp32, kind="ExternalInput").ap()
    wq_d = nc.dram_tensor("wq", [HL * HD, D], fp32, kind="ExternalInput").ap()
    wk_d = nc.dram_tensor("wk", [2 * HD, D], fp32, kind="ExternalInput").ap()
    wv_d = nc.dram_tensor("wv", [2 * HD, D], fp32, kind="ExternalInput").ap()
    wp_d = nc.dram_tensor("wp", [D, HL * HD], fp32, kind="ExternalInput").ap()
    cos_d = nc.dram_tensor("cos", [T, HD // 2], fp32, kind="ExternalInput").ap()
    sin_d = nc.dram_tensor("sin", [T, HD // 2], fp32, kind="ExternalInput").ap()
    gain_d = nc.dram_tensor("gain", [P, HL], fp32, kind="ExternalInput").ap()
    out_d = nc.dram_tensor("out", [T, D], fp32, kind="ExternalOutput").ap()

    x3 = x_d.rearrange("(n p) d -> n p d", p=P)
    out3 = out_d.rearrange("(n p) d -> n p d", p=P)

    with tile.TileContext(nc) as tc:
        with (
            tc.tile_pool(name="persist", bufs=1) as persist,
            tc.tile_pool(name="p_pool", bufs=2) as pp,
            tc.tile_pool(name="yT_pool", bufs=2) as ytp,
            tc.tile_pool(name="stage_e", bufs=2) as se,
            tc.tile_pool(name="small", bufs=8) as sm,
            tc.tile_pool(name="ps_att", bufs=2, space="PSUM") as psat,
            tc.tile_pool(name="ps_y", bufs=2, space="PSUM") as psy,
        ):
            # ---- constants ----
            ident = persist.tile([P, P], bf16)
            make_identity(nc, ident)
            identf = persist.tile([P, P], fp32)
            make_identity(nc, identf)
            dmask = persist.tile([P, P], bf16)
            make_upper_triangular(nc, dmask, val=1.0, diag=True)
            cos_sb = persist.tile([P, NT, HD // 2], bf16)
            sin_sb = persist.tile([P, NT, HD // 2], bf16)
            nc.gpsimd.dma_start(cos_sb, cos_d.rearrange("(n p) c -> p n c", p=P))
            nc.gpsimd.dma_start(sin_sb, sin_d.rearrange("(n p) c -> p n c", p=P))
            gain_sb = persist.tile([P, HL], fp32)
            nc.sync.dma_start(gain_sb, gain_d)
            eps_sb = persist.tile([P, 1], fp32)
            nc.vector.memset(eps_sb, EPS)

            # ---- persistent activations / weights ----
            qT = persist.tile([P, PAIRS, T], bf16)   # [2-head dims, pair, t]
            kT2 = persist.tile([P, 2, T], bf16)      # kv heads, replicated halves
            v_sb = persist.tile([P, NT, 2 * (HD + 1)], bf16)  # ones-augmented
            y_nat = persist.tile([P, NT, HL * HD], bf16)
            wpT = persist.tile([P, PAIRS, D], bf16)
            wqT = persist.tile([P, DC, HL * HD], bf16)
            wkvT = persist.tile([P, DC, 4 * HD], bf16)

            v4 = v_sb.rearrange("p n (h x) -> p n h x", h=2)
            nc.gpsimd.memset(v4[:, :, :, HD : HD + 1], 1.0)

            def transpose_in(src_f32, n_chunk, dst, dst_col0, ncols, pool,
                             copy_eng=None):
                # src [P, n_chunk*128] f32 -> dst[:, c, col0:+ncols] bf16
                for g0 in range(0, n_chunk, 4):
                    gn = min(4, n_chunk - g0)
                    ps = pool.tile([P, 512], fp32, tag="m")
                    for gi in range(gn):
                        c = g0 + gi
                        nc.tensor.transpose(
                            ps[:, gi * P : (gi + 1) * P],
                            src_f32[:, c * P : (c + 1) * P],
                            identf,
                        )
                    if copy_eng == "scalar":
                        nc.scalar.copy(
                            dst[:, g0 : g0 + gn, dst_col0 : dst_col0 + ncols],
                            ps.rearrange("p (a b) -> p a b", b=P)[:, :gn, :ncols],
                        )
                    else:
                        nc.vector.tensor_copy(
                            dst[:, g0 : g0 + gn, dst_col0 : dst_col0 + ncols],
                            ps.rearrange("p (a b) -> p a b", b=P)[:, :gn, :ncols],
                        )

            # ================= weights =================
            with (
                tc.tile_pool(name="stage_w", bufs=3) as stw,
                tc.tile_pool(name="ps_w", bufs=2, space="PSUM") as psw,
            ):
                for rt in range(4):  # wq rows: e = rt*128 + p
                    wc = stw.tile([P, D], fp32, tag="wc")
                    nc.sync.dma_start(
                        wc, wq_d.rearrange("(r p) d -> r p d", p=P)[rt]
                    )
                    transpose_in(wc, DC, wqT, rt * P, P, psw)
                wc = stw.tile([P, D], fp32, tag="wc")
                nc.sync.dma_start(wc, wk_d)
                transpose_in(wc, DC, wkvT, 0, P, psw)
                wc = stw.tile([P, D], fp32, tag="wc")
                nc.sync.dma_start(wc, wv_d)
                transpose_in(wc, DC, wkvT, 2 * HD, P, psw)

            # ===== phase C tiles interleaved with attention strips =====
            # Emission (and so each engine's static order) alternates four
            # QKV tiles with the strip they complete, so strip g's scores/
            # exp start as soon as q^T/k^T tiles 0..4g+3 exist. Projections
            # are deferred past phase C so their PSUM can reuse its banks.
            with (
                tc.tile_pool(name="stage_c", bufs=2) as sc_st,
                tc.tile_pool(name="xf_pool", bufs=4) as xfp,
                tc.tile_pool(name="ps_cqkv", bufs=1, space="PSUM") as psqkv,
                tc.tile_pool(name="ps_cm", bufs=1, space="PSUM") as pscm,
            ):

                def norm_rope(nt, src_ps, nh, gain):
                    # src_ps [P, nh*HD] fp32 psum -> roped+normed bf16
                    sb = sc_st.tile([P, nh * HD], bf16, tag=f"sb{nh}")
                    nc.vector.tensor_copy(sb, src_ps)
                    s3 = sb.rearrange("p (h x) -> p h x", h=nh)
                    sq = sc_st.tile([P, nh * HD], bf16, tag=f"sq{nh}")
                    nc.vector.tensor_tensor(sq, sb, sb, MUL)
                    ss = sc_st.tile([P, nh], fp32, tag=f"ss{nh}")
                    nc.vector.reduce_sum(
                        ss, sq.rearrange("p (h x) -> p h x", h=nh), axis=AX
                    )
                    rms = sc_st.tile([P, nh], fp32, tag=f"rm{nh}")
                    nc.scalar.activation(
                        rms, ss, SQRT, bias=eps_sb[:, 0:1], scale=1.0 / HD
                    )
                    inv = sc_st.tile([P, nh], fp32, tag=f"iv{nh}")
                    nc.vector.reciprocal(inv, rms)
                    if gain is not None:
                        nc.vector.tensor_tensor(inv, inv, gain, MUL)
                    h2 = HD // 2
                    x1 = s3[:, :, 0:h2]
                    x2 = s3[:, :, h2:HD]
                    cb = cos_sb[:, nt : nt + 1, :].to_broadcast([P, nh, h2])
                    sbr = sin_sb[:, nt : nt + 1, :].to_broadcast([P, nh, h2])
                    r = sc_st.tile([P, nh * HD], bf16, tag=f"r{nh}")
                    r3 = r.rearrange("p (h x) -> p h x", h=nh)
                    tmp = sc_st.tile([P, nh * (HD // 2)], bf16, tag=f"t{nh}")
                    t3 = tmp.rearrange("p (h x) -> p h x", h=nh)
                    nc.vector.tensor_tensor(r3[:, :, 0:h2], x1, cb, MUL)
                    nc.vector.tensor_tensor(t3, x2, sbr, MUL)
                    nc.vector.tensor_tensor(
                        r3[:, :, 0:h2], r3[:, :, 0:h2], t3, ADD
                    )
                    nc.vector.tensor_tensor(r3[:, :, h2:HD], x2, cb, MUL)
                    nc.vector.tensor_tensor(t3, x1, sbr, MUL)
                    nc.vector.tensor_tensor(
                        r3[:, :, h2:HD], r3[:, :, h2:HD], t3, SUB
                    )
                    ivb = inv[:, :, None].to_broadcast([P, nh, HD])
                    nc.vector.tensor_tensor(r3, r3, ivb, MUL)
                    return r

                def tile_c(nt):
                    xf = xfp.tile([P, D], fp32, tag="xf")
                    nc.sync.dma_start(xf, x3[nt])
                    xTt = sc_st.tile([P, DC, P], bf16, tag="xT")
                    transpose_in(xf, DC, xTt, 0, P, pscm)

                    q_ps = psqkv.tile([P, HL * HD], fp32, tag="qkv")
                    for dc in range(DC):
                        nc.tensor.matmul(
                            q_ps, xTt[:, dc, :], wqT[:, dc, :],
                            start=(dc == 0), stop=(dc == DC - 1),
                        )
                    kv_full = psqkv.tile([P, HL * HD], fp32, tag="qkv")
                    for dc in range(DC):
                        nc.tensor.matmul(
                            kv_full[:, 0 : 4 * HD], xTt[:, dc, :], wkvT[:, dc, :],
                            start=(dc == 0), stop=(dc == DC - 1),
                        )
                    k_ps = kv_full[:, 0 : 2 * HD]
                    v_ps = kv_full[:, 2 * HD : 4 * HD]
                    nc.vector.tensor_copy(
                        v4[:, nt, :, 0:HD],
                        v_ps.rearrange("p (h x) -> p h x", h=2),
                    )
                    kr = norm_rope(nt, k_ps, 2, None)
                    qr = norm_rope(nt, q_ps, HL, gain_sb)

                    # q^T: 4 pair transposes
                    ps = pscm.tile([P, 512], bf16, tag="m")
                    for pr in range(PAIRS):
                        nc.tensor.transpose(
                            ps[:, pr * P : (pr + 1) * P],
                            qr[:, pr * P : (pr + 1) * P],
                            ident,
                        )
                    nc.vector.tensor_copy(
                        qT[:, :, nt * P : (nt + 1) * P],
                        ps.rearrange("p (a b) -> p a b", b=P),
                    )
                    # k^T replicated to both partition halves
                    kps = pscm.tile([P, 2, P], bf16, tag="m")
                    for kv in range(2):
                        for rep in range(2):
                            nc.tensor.transpose(
                                kps[rep * 64 : (rep + 1) * 64, kv, :],
                                kr[:, kv * HD : (kv + 1) * HD],
                                ident,
                                tile_position=(0, rep * 64),
                            )
                    nc.vector.tensor_copy(
                        kT2[:, :, nt * P : (nt + 1) * P], kps
                    )

                def emit_scores(s, pr):
                    tq0 = s * 512
                    kv = pr // 2
                    p_tiles = {}
                    for tkb in range(4 * s + 4):
                        m = tkb - 4 * s  # >=0 only for diagonal-strip blocks
                        c0 = max(m, 0) * P   # first causally-valid strip col
                        pt = pp.tile([P, 2, 512], bf16, tag=f"p{tkb}")
                        p_tiles[tkb] = pt
                        sc = psat.tile([P, 2, 512], fp32, tag="sc")
                        for h01 in range(2):
                            hp = h01 * 64
                            nc.tensor.matmul(
                                sc[:, h01, c0:512],
                                kT2[hp : hp + 64, kv, tkb * P : (tkb + 1) * P],
                                qT[hp : hp + 64, pr, tq0 + c0 : tq0 + 512],
                                start=True, stop=True,
                            )
                        nc.scalar.activation(
                            pt[:, :, c0:512], sc[:, :, c0:512], EXP, scale=SCALE
                        )
                        if m >= 0:
                            # triangular mask on the diagonal 128-block
                            # (gpsimd: idle engine, keeps DVE free)
                            dm = dmask[:, None, :].to_broadcast([P, 2, P])
                            nc.gpsimd.tensor_tensor(
                                pt[:, :, c0 : c0 + P],
                                pt[:, :, c0 : c0 + P], dm, MUL,
                            )
                    return p_tiles

                def emit_av(s, pr, p_tiles):
                    kv = pr // 2
                    for tqi in range(4 * s, 4 * s + 4):
                        co = (tqi - 4 * s) * P
                        for h01 in range(2):
                            y_ps = psy.tile([P, HD + 1], fp32, tag="y")
                            for tkb in range(tqi + 1):
                                nc.tensor.matmul(
                                    y_ps,
                                    p_tiles[tkb][:, h01, co : co + P],
                                    v_sb[:, tkb,
                                         kv * (HD + 1) : (kv + 1) * (HD + 1)],
                                    start=(tkb == 0), stop=(tkb == tqi),
                                )
                            rcp = sm.tile([P, 1], fp32, tag="rcp")
                            nc.vector.reciprocal(rcp, y_ps[:, HD : HD + 1])
                            h = 2 * pr + h01
                            nc.vector.tensor_tensor(
                                y_nat[:, tqi, h * HD : (h + 1) * HD],
                                y_ps[:, 0:HD],
                                rcp[:, 0:1].to_broadcast([P, HD]),
                                MUL,
                            )

                import os as _os
                _interleave = _os.environ.get("K_EMIT", "flat") == "interleave"
                if _interleave:
                    # Global software pipeline: every AV(g,pr) (gated on its
                    # exps) is preceded in emission by independent PE work.
                    for nt in range(4):
                        tile_c(nt)
                    for g in range(3):
                        tiles = {0: emit_scores(g, 0)}
                        tiles[1] = emit_scores(g, 1)
                        emit_av(g, 0, tiles.pop(0))
                        tile_c(4 * g + 4)
                        tiles[2] = emit_scores(g, 2)
                        emit_av(g, 1, tiles.pop(1))
                        tile_c(4 * g + 5)
                        tiles[3] = emit_scores(g, 3)
                        emit_av(g, 2, tiles.pop(2))
                        tile_c(4 * g + 6)
                        tile_c(4 * g + 7)
                        emit_av(g, 3, tiles.pop(3))
                else:
                    for nt in range(NT):
                        tile_c(nt)
                    for g in range(3):
                        tiles_cur = emit_scores(g, 0)
                        for pr in range(PAIRS):
                            tiles_next = (
                                emit_scores(g, pr + 1) if pr + 1 < PAIRS else None
                            )
                            emit_av(g, pr, tiles_cur)
                            tiles_cur = tiles_next

            # wp^T transposes (feed only the projection)
            with (
                tc.tile_pool(name="stage_w2", bufs=2) as stw2,
                tc.tile_pool(name="ps_w2", bufs=1, space="PSUM") as psw2,
            ):
                for rt in range(DC):  # wp rows: e = rt*128 + p
                    wc = stw2.tile([P, HL * HD], fp32, tag="wpc")
                    nc.sync.dma_start(
                        wc, wp_d.rearrange("(r p) d -> r p d", p=P)[rt]
                    )
                    transpose_in(wc, PAIRS, wpT, rt * P, P, psw2)

            # ===== strip 3 interleaved with all projections =====
            with tc.tile_pool(name="ps_e", bufs=1, space="PSUM") as pse:

                def proj(s):
                    yTs = ytp.tile([P, PAIRS, 512], bf16, tag="yT")
                    for j in range(4):
                        nt = 4 * s + j
                        ps = pse.tile([P, 512], bf16, tag="yt")
                        for prr in range(PAIRS):
                            nc.tensor.transpose(
                                ps[:, prr * P : (prr + 1) * P],
                                y_nat[:, nt, prr * P : (prr + 1) * P],
                                ident,
                            )
                        nc.vector.tensor_copy(
                            yTs[:, :, j * P : (j + 1) * P],
                            ps.rearrange("p (a b) -> p a b", b=P),
                        )
                    for j in range(4):
                        nt = 4 * s + j
                        o_sb = se.tile([P, D], fp32, tag="osb")
                        for ec in range(2):
                            o_ps = pse.tile([P, 512], fp32, tag="o")
                            for prr in range(PAIRS):
                                nc.tensor.matmul(
                                    o_ps,
                                    yTs[:, prr, j * P : (j + 1) * P],
                                    wpT[:, prr, ec * 512 : (ec + 1) * 512],
                                    start=(prr == 0), stop=(prr == PAIRS - 1),
                                )
                            nc.vector.tensor_copy(
                                o_sb[:, ec * 512 : (ec + 1) * 512], o_ps
                            )
                        nc.sync.dma_start(out3[nt], o_sb)

                t30 = emit_scores(3, 0)
                t31 = emit_scores(3, 1)
                emit_av(3, 0, t30)
                proj(0)
                t32 = emit_scores(3, 2)
                emit_av(3, 1, t31)
                proj(1)
                t33 = emit_scores(3, 3)
                emit_av(3, 2, t32)
                proj(2)
                emit_av(3, 3, t33)
                proj(3)

    nc.compile()
    return nc


def _get_program():
    if "nc" not in _CACHE:
        _CACHE["nc"] = _build_program()
    return _CACHE["nc"]


def make_in_maps(x, Wq, Wk, Wv, Wproj, q_gain):
    cos, sin = _rope_tables()
    in_maps = []
    for c in range(8):
        b, hh = c // 2, c % 2
        in_maps.append(
            {
                "x": np.ascontiguousarray(x[b]),
                "wq": np.ascontiguousarray(Wq[hh * 512 : (hh + 1) * 512]),
                "wk": np.ascontiguousarray(Wk[hh * 128 : (hh + 1) * 128]),
                "wv": np.ascontiguousarray(Wv[hh * 128 : (hh + 1) * 128]),
                "wp": np.ascontiguousarray(Wproj[:, hh * 512 : (hh + 1) * 512]),
                "cos": cos,
                "sin": sin,
                "gain": np.ascontiguousarray(
                    np.broadcast_to(q_gain[hh * 8 : (hh + 1) * 8], (P, HL))
                ),
            }
        )
    return in_maps


def kernel(x, Wq, Wk, Wv, Wproj, q_gain):
    from concourse import bass_utils

    x = np.asarray(x, dtype=np.float32)
    Wq = np.asarray(Wq, dtype=np.float32)
    Wk = np.asarray(Wk, dtype=np.float32)
    Wv = np.asarray(Wv, dtype=np.float32)
    Wproj = np.asarray(Wproj, dtype=np.float32)
    q_gain = np.asarray(q_gain, dtype=np.float32)

    nc = _get_program()
    in_maps = make_in_maps(x, Wq, Wk, Wv, Wproj, q_gain)
    res = bass_utils.run_bass_kernel_spmd(
        nc, in_maps, core_ids=list(range(8)), trace=False
    )
    out = np.empty((B, T, D), dtype=np.float32)
    for b in range(B):
        out[b] = res.results[2 * b]["out"] + res.results[2 * b + 1]["out"]
    return out
